# revision 1
# baseline (speedup 1.0000x reference)
"""CrossAttn + TISA bias kernel for TRN2, 8-core SPMD.

Sharding: core = (batch b = core//2, query half = core%2).
Each core computes the full kv projection for its batch (duplicated within
the pair) and its 512 query rows end-to-end. No collectives.

Inputs arrive host-transposed: xqt/xkvt are [d_in, tokens].
  qT:   [d_out(part), i]   (scaled by 1/sqrt(Dh))
  kT:   [d_out(part), j]
  v:    [j(part), d_out]
  S^T:  [j(part), i] = kT_h.T @ qT_h          (K=64)
  wT = exp(S^T) * srow[:, C:C+512]            (shifted exp-bias table slice)
  sums: mask-matmul E2.T @ wT -> psum [2, 512]
  attn: v_h.T @ wT -> psum chunk rows {0:64, 64:128} per head pair
  attn_norm = attn * bcast(1/sums)            (PE broadcast of recip)
  gate: attn_norm.T @ Wg -> [i(part), 2048]; out = (a+bga)*sigmoid(b+bgb)
"""

import numpy as np
import ml_dtypes

import concourse.bacc as bacc
import concourse.mybir as mybir
import concourse.tile as tile
from concourse.bass import ts

L = 1024
D = 1024
H = 16
DH = 64
LQ = 512          # q rows per core
NIC = LQ // 128   # 4 i-chunks
NJC = L // 128    # 8 j-chunks
NKC = D // 128    # 8 d_model chunks
SROW_W = 1408
NUM_KERNELS = 21

F32 = mybir.dt.float32
EXP = mybir.ActivationFunctionType.Exp
SIG = mybir.ActivationFunctionType.Sigmoid
CPY = mybir.ActivationFunctionType.Copy
MUL = mybir.AluOpType.mult
ADD = mybir.AluOpType.add

_DT = {"f32": mybir.dt.float32, "bf16": mybir.dt.bfloat16}
_NP = {"f32": np.float32, "bf16": ml_dtypes.bfloat16}


def ds2(hh):
    return slice(hh * 64, hh * 64 + 64)


def build_nc(cfg="bf16"):
    mdt = _DT[cfg]
    sdt = mdt   # srow/wexp dtype rides the matmul dtype
    resident = cfg == "bf16"

    nc = bacc.Bacc("TRN2", target_bir_lowering=False, debug=False, num_devices=8)

    xqt_d = nc.dram_tensor("xqt", [D, LQ], mdt, kind="ExternalInput").ap()
    xkvt_d = nc.dram_tensor("xkvt", [D, L], mdt, kind="ExternalInput").ap()
    wq_d = nc.dram_tensor("wq", [D, D], mdt, kind="ExternalInput").ap()
    wm_d = nc.dram_tensor("wm", [D, 2 * D], mdt, kind="ExternalInput").ap()
    wg_d = nc.dram_tensor("wg", [D, 2 * D], mdt, kind="ExternalInput").ap()
    srow_d = nc.dram_tensor("srow", [H, 128, SROW_W], sdt, kind="ExternalInput").ap()
    bg_d = nc.dram_tensor("bgrep", [128, 2 * D], F32, kind="ExternalInput").ap()
    e2_d = nc.dram_tensor("e2", [128, 4], mdt, kind="ExternalInput").ap()
    p2_d = nc.dram_tensor("p2", [2, 128], F32, kind="ExternalInput").ap()
    out_d = nc.dram_tensor("out", [LQ, D], F32, kind="ExternalOutput").ap()

    with tile.TileContext(nc) as tc:
        with (
            tc.tile_pool(name="const", bufs=1) as constp,
            tc.tile_pool(name="persist", bufs=1) as pers,
            tc.tile_pool(name="psum", bufs=1, space="PSUM") as psum,
            tc.tile_pool(name="phB", bufs=1) as phb,
            tc.tile_pool(name="phC", bufs=1) as phc,
            tc.tile_pool(name="phD", bufs=1) as phd,
        ):
            e2_sb = constp.tile([128, 4], mdt)
            nc.sync.dma_start(out=e2_sb, in_=e2_d)
            p2_sb = constp.tile([2, 128], F32)
            nc.sync.dma_start(out=p2_sb, in_=p2_d)
            bg_sb = constp.tile([128, 2 * D], F32)
            nc.sync.dma_start(out=bg_sb, in_=bg_d)

            qT = pers.tile([128, NKC, LQ], mdt)        # [d_out, mc, i]
            kT = pers.tile([128, NKC, L], mdt)         # [d_out, mc, j]
            vsb = pers.tile([128, NJC, D], mdt)        # [j, jc, d_out]
            attn = pers.tile([128, NKC, LQ], mdt)      # [d_model, chunk, i]

            # =========== phase B: projections ==========
            if True:
                xqT = phb.tile([128, NKC, LQ], mdt)    # [d_in, kc, i]
                xkvT = phb.tile([128, NKC, L], mdt)    # [d_in, kc, j]
                for kc in range(NKC):
                    nc.sync.dma_start(out=xqT[:, kc, :], in_=xqt_d[ts(kc, 128), :])
                    nc.sync.dma_start(out=xkvT[:, kc, :], in_=xkvt_d[ts(kc, 128), :])

                if resident:
                    wq_r = phb.tile([128, NKC, D], mdt)
                    wm_r = phb.tile([128, NKC, 2 * D], mdt)
                    for kc in range(NKC):
                        nc.sync.dma_start(out=wq_r[:, kc, :], in_=wq_d[ts(kc, 128), :])
                        nc.sync.dma_start(out=wm_r[:, kc, :], in_=wm_d[ts(kc, 128), :])

                def get_wq(kc, cols):
                    if resident:
                        return wq_r[:, kc, cols]
                    t = phb.tile([128, 128], mdt, tag="wqs", bufs=3)
                    nc.sync.dma_start(out=t, in_=wq_d[ts(kc, 128), cols])
                    return t

                def get_wm(kc, cols, n):
                    if resident:
                        return wm_r[:, kc, cols]
                    t = phb.tile([128, n], mdt, tag=f"wms{n}", bufs=3)
                    nc.sync.dma_start(out=t, in_=wm_d[ts(kc, 128), cols])
                    return t

                for mc in range(NKC):
                    ps = psum.tile([128, LQ], F32, tag="t1", bufs=4)
                    for kc in range(NKC):
                        nc.tensor.matmul(
                            ps, get_wq(kc, ts(mc, 128)), xqT[:, kc, :],
                            start=(kc == 0), stop=(kc == NKC - 1))
                    nc.scalar.activation(qT[:, mc, :], ps, CPY, scale=0.125)

                for mc in range(NKC):
                    for nh in range(2):
                        ps = psum.tile([128, 512], F32, tag="t1", bufs=4)
                        for kc in range(NKC):
                            nc.tensor.matmul(
                                ps, get_wm(kc, ts(mc, 128), 128),
                                xkvT[:, kc, ts(nh, 512)],
                                start=(kc == 0), stop=(kc == NKC - 1))
                        if nh == 0:
                            nc.vector.tensor_copy(kT[:, mc, ts(nh, 512)], ps)
                        else:
                            nc.scalar.activation(kT[:, mc, ts(nh, 512)], ps, CPY)

                for jc in range(NJC):
                    for nh in range(2):
                        ps = psum.tile([128, 512], F32, tag="t1", bufs=4)
                        for kc in range(NKC):
                            nc.tensor.matmul(
                                ps, xkvT[:, kc, ts(jc, 128)],
                                get_wm(kc, slice(D + nh * 512, D + nh * 512 + 512), 512),
                                start=(kc == 0), stop=(kc == NKC - 1))
                        if nh == 0:
                            nc.vector.tensor_copy(vsb[:, jc, ts(nh, 512)], ps)
                        else:
                            nc.scalar.activation(vsb[:, jc, ts(nh, 512)], ps, CPY)

            # ================= phase C: attention =================
            if True:
                for c in range(NKC):
                    ps_at = psum.tile([128, LQ], F32, tag="attn", bufs=2)
                    ps_sum = psum.tile([2, LQ], F32, tag="sums", bufs=2)
                    for hh in range(2):
                        h = 2 * c + hh
                        srow_sb = phc.tile([128, SROW_W], sdt, tag="srow", bufs=2)
                        nc.sync.dma_start(out=srow_sb, in_=srow_d[h, :, :])
                        wts = []
                        for jc in range(NJC):
                            ps_s = psum.tile([128, LQ], F32, tag="t1", bufs=4)
                            nc.tensor.matmul(
                                ps_s, kT[ds2(hh), c, ts(jc, 128)], qT[ds2(hh), c, :],
                                start=True, stop=True)
                            wexp = phc.tile([128, LQ], sdt, tag="wexp", bufs=4)
                            nc.scalar.activation(wexp, ps_s, EXP)
                            wT = phc.tile([128, LQ], mdt, tag="wt", bufs=8)
                            C0 = 896 - jc * 128
                            nc.vector.tensor_tensor(
                                wT, wexp, srow_sb[:, C0:C0 + LQ], MUL)
                            wts.append(wT)
                        for jc in range(NJC):
                            nc.tensor.matmul(
                                ps_sum, e2_sb[:, 2 * hh:2 * hh + 2], wts[jc],
                                start=(hh == 0 and jc == 0),
                                stop=(hh == 1 and jc == NJC - 1))
                            nc.tensor.matmul(
                                ps_at[ds2(hh), :], vsb[:, jc, ts(h, DH)], wts[jc],
                                start=(jc == 0), stop=(jc == NJC - 1))
                    rsum = phc.tile([2, LQ], F32, tag="rsum", bufs=2)
                    nc.vector.reciprocal(rsum, ps_sum)
                    ps_rb = psum.tile([128, LQ], F32, tag="t1", bufs=4)
                    nc.tensor.matmul(ps_rb, p2_sb, rsum, start=True, stop=True)
                    rb = phc.tile([128, LQ], F32, tag="rb", bufs=2)
                    nc.scalar.activation(rb, ps_rb, CPY)
                    nc.vector.tensor_tensor(attn[:, c, :], ps_at, rb, MUL)

            # ================= phase D: gate =================
            if True:
                if resident:
                    wg_r = phd.tile([128, NKC, 2 * D], mdt)
                    for kc in range(NKC):
                        nc.sync.dma_start(out=wg_r[:, kc, :], in_=wg_d[ts(kc, 128), :])

                def get_wg(kc, cols):
                    if resident:
                        return wg_r[:, kc, cols]
                    t = phd.tile([128, 512], mdt, tag="wgs", bufs=3)
                    nc.sync.dma_start(out=t, in_=wg_d[ts(kc, 128), cols])
                    return t

                for ic in range(NIC):
                    out_t = phd.tile([128, D], F32, tag="outt", bufs=2)
                    for qa in range(2):
                        ps_a = psum.tile([128, 512], F32, tag="t1", bufs=4)
                        ps_b = psum.tile([128, 512], F32, tag="t1", bufs=4)
                        for kc in range(NKC):
                            nc.tensor.matmul(
                                ps_a, attn[:, kc, ts(ic, 128)], get_wg(kc, ts(qa, 512)),
                                start=(kc == 0), stop=(kc == NKC - 1))
                        for kc in range(NKC):
                            nc.tensor.matmul(
                                ps_b, attn[:, kc, ts(ic, 128)],
                                get_wg(kc, slice(D + qa * 512, D + qa * 512 + 512)),
                                start=(kc == 0), stop=(kc == NKC - 1))
                        ta = phd.tile([128, 512], F32, tag="ta", bufs=2)
                        nc.vector.tensor_tensor(ta, ps_a, bg_sb[:, ts(qa, 512)], ADD)
                        tb = phd.tile([128, 512], F32, tag="tb", bufs=2)
                        nc.vector.tensor_tensor(
                            tb, ps_b, bg_sb[:, D + qa * 512:D + qa * 512 + 512], ADD)
                        tsg = phd.tile([128, 512], F32, tag="tsg", bufs=2)
                        nc.scalar.activation(tsg, tb, SIG)
                        nc.vector.tensor_tensor(out_t[:, ts(qa, 512)], ta, tsg, MUL)
                    nc.sync.dma_start(out=out_d[ts(ic, 128), :], in_=out_t)

    nc.compile()
    return nc


# ======================= host side =======================

def _tisa_ebias(amp, off, sharp):
    d = np.arange(-(L - 1), L, dtype=np.float32)
    s = np.sum(
        amp[:, :, None].astype(np.float32)
        * np.exp(-np.abs(sharp)[:, :, None].astype(np.float32)
                 * (d[None, None, :] - off[:, :, None].astype(np.float32)) ** 2),
        axis=1, dtype=np.float32).astype(np.float32)
    return np.exp(s).astype(np.float32)


def make_host_inputs(inputs, cfg="bf16"):
    npdt = _NP[cfg]
    x_q = np.asarray(inputs["x_q"])
    x_kv = np.asarray(inputs["x_kv"])
    wq = np.asarray(inputs["Wq"]).astype(npdt)
    wm = np.asarray(inputs["Wm"]).astype(npdt)
    wg = np.asarray(inputs["Wg"]).astype(npdt)
    bg = np.asarray(inputs["bg"]).astype(np.float32)

    ebias = _tisa_ebias(np.asarray(inputs["tisa_amp"]),
                        np.asarray(inputs["tisa_off"]),
                        np.asarray(inputs["tisa_sharp"]))

    p_i = np.arange(128)[:, None]
    m_i = np.arange(SROW_W)[None, :]
    srows = []
    for i_off in (0, 512):
        idx = p_i - m_i + (1919 - i_off)
        srows.append(np.ascontiguousarray(ebias[:, idx]).astype(npdt))

    e2 = np.zeros((128, 4), dtype=npdt)
    e2[:, 0] = 1
    e2[:, 3] = 1
    p2 = np.zeros((2, 128), dtype=np.float32)
    p2[0, :64] = 1
    p2[1, 64:] = 1
    bgrep = np.ascontiguousarray(np.broadcast_to(bg, (128, 2 * D))).astype(np.float32)

    in_maps = []
    for core in range(8):
        b, half = core // 2, core % 2
        in_maps.append({
            "xqt": np.ascontiguousarray(
                x_q[b, half * LQ:(half + 1) * LQ].T).astype(npdt),
            "xkvt": np.ascontiguousarray(x_kv[b].T).astype(npdt),
            "wq": wq, "wm": wm, "wg": wg,
            "srow": srows[half],
            "bgrep": bgrep, "e2": e2, "p2": p2,
        })
    return in_maps


def assemble_output(results):
    out = np.empty((4, L, D), dtype=np.float32)
    for core in range(8):
        b, half = core // 2, core % 2
        out[b, half * LQ:(half + 1) * LQ] = results[core]["out"]
    return out


# ======================= public entry point =======================

_NC_CACHE = {}


def _get_nc(cfg):
    if cfg not in _NC_CACHE:
        _NC_CACHE[cfg] = build_nc(cfg)
    return _NC_CACHE[cfg]


def kernel(**inputs):
    """Full (unsharded) inputs -> full (4, 1024, 1024) float32 output.

    Shards over 8 NeuronCores: core = (batch, query-half). Host precomputes
    the TISA exp-bias lookup table and pre-transposes activations; all dense
    compute (projections, attention, gate) runs on-device in bf16 matmuls
    with fp32 accumulation.
    """
    from concourse.bass_utils import run_bass_kernel_spmd

    cfg = "bf16"
    nc = _get_nc(cfg)
    in_maps = make_host_inputs(inputs, cfg)
    res = run_bass_kernel_spmd(nc, in_maps, core_ids=list(range(8)))
    return assemble_output(res.results)



# revision 2
# speedup vs baseline: 1.0083x; 1.0083x over previous
"""CrossAttn + TISA bias kernel for TRN2, 8-core SPMD.

Sharding: core = (batch b = core//2, query half = core%2).
Each core computes the full kv projection for its batch (duplicated within
the pair) and its 512 query rows end-to-end. No collectives.

v2 vs v1:
  - softmax denominator fused into the attn matmul: stationary [v_h | 1]
    (M=65) so each wT tile streams through the PE once, not twice
  - per-head reciprocal broadcast via a K=1 ones matmul (bf16)
  - odd heads' normalized attn moved to partitions 64..127 with a small
    SBUF->SBUF DMA (engines cannot shift partitions)
  - weight DMAs reordered/column-chunked so the first q-proj matmul starts
    after ~0.4 MB instead of 9 MB
  - gate phase emits 512-col output halves, sigmoid path first, so the
    tail after the last matmul is short

Inputs arrive host-transposed: xqt/xkvt are [d_in, tokens].
  qT:   [d_out(part), i]   (scaled by 1/sqrt(Dh))
  kT:   [d_out(part), j]
  vaug: [j(part), jc, h, 65] = v columns 0..63, ones column 64
  S^T:  [j(part), i] = kT_h.T @ qT_h          (K=64)
  wT = exp(S^T) * srow[:, C:C+512]            (shifted exp-bias table slice)
  ps_h = [vaug_h]^T @ wT accumulated over jc  -> rows 0..63 attn, row 64 sums
  rb = ones1^T @ (1/sums)                     (K=1 PE broadcast)
  attn = ps_h * rb                            (per-head normalize)
  gate: attn.T @ Wg -> [i(part), 2048]; out = (a+bga)*sigmoid(b+bgb)
"""

import numpy as np
import ml_dtypes

import concourse.bacc as bacc
import concourse.mybir as mybir
import concourse.tile as tile
from concourse.bass import ts

L = 1024
D = 1024
H = 16
DH = 64
LQ = 512          # q rows per core
NIC = LQ // 128   # 4 i-chunks
NJC = L // 128    # 8 j-chunks
NKC = D // 128    # 8 d_model chunks
SROW_W = 1408
NUM_KERNELS = 21

F32 = mybir.dt.float32
BF16 = mybir.dt.bfloat16
EXP = mybir.ActivationFunctionType.Exp
SIG = mybir.ActivationFunctionType.Sigmoid
CPY = mybir.ActivationFunctionType.Copy
MUL = mybir.AluOpType.mult
ADD = mybir.AluOpType.add

_NP = {"f32": np.float32, "bf16": ml_dtypes.bfloat16}


def ds2(hh):
    return slice(hh * 64, hh * 64 + 64)


def build_nc(cfg="bf16"):
    mdt = BF16

    nc = bacc.Bacc("TRN2", target_bir_lowering=False, debug=False, num_devices=8)

    xqt_d = nc.dram_tensor("xqt", [D, LQ], mdt, kind="ExternalInput").ap()
    xkvt_d = nc.dram_tensor("xkvt", [D, L], mdt, kind="ExternalInput").ap()
    wqc_d = nc.dram_tensor("wqc", [NKC, D, 128], mdt, kind="ExternalInput").ap()
    wmk_d = nc.dram_tensor("wmk", [NKC, D, 128], mdt, kind="ExternalInput").ap()
    wmv_d = nc.dram_tensor("wmv", [D, D], mdt, kind="ExternalInput").ap()
    wg_d = nc.dram_tensor("wg", [D, 2 * D], mdt, kind="ExternalInput").ap()
    srow_d = nc.dram_tensor("srow", [H, 128, SROW_W], mdt, kind="ExternalInput").ap()
    bg_d = nc.dram_tensor("bgrep", [128, 2 * D], F32, kind="ExternalInput").ap()
    out_d = nc.dram_tensor("out", [LQ, D], F32, kind="ExternalOutput").ap()

    with tile.TileContext(nc) as tc:
        with (
            tc.tile_pool(name="const", bufs=1) as constp,
            tc.tile_pool(name="persist", bufs=1) as pers,
            tc.tile_pool(name="psum", bufs=1, space="PSUM") as psum,
        ):
            onesc = constp.tile([128, 64], mdt)
            nc.gpsimd.memset(onesc, 1.0)

            qT = pers.tile([128, NKC, LQ], mdt)        # [d_out, mc, i]
            kT = pers.tile([128, NKC, L], mdt)         # [d_out, mc, j]
            vaug = pers.tile([128, NJC, H, 65], mdt)   # [j, jc, h, v|1]
            attn = pers.tile([128, NKC, LQ], mdt)      # [d_model, chunk, i]

            # =========== phase B: projections ==========
            with tc.tile_pool(name="phB", bufs=1) as phb:
                wqc = phb.tile([128, NKC, NKC, 128], mdt)   # [k, mc, kc, col]
                xqT = phb.tile([128, NKC, LQ], mdt)         # [d_in, kc, i]
                xkvT = phb.tile([128, NKC, L], mdt)         # [d_in, kc, j]
                wmk = phb.tile([128, NKC, NKC, 128], mdt)
                wmv = phb.tile([128, NKC, D], mdt)          # [k, kc, v-cols]

                # DMA order = need order: q-proj can start after wqc[0]+xqT[0]
                nc.sync.dma_start(
                    out=wqc[:, 0], in_=wqc_d[0].rearrange("(kc p) c -> p kc c", p=128))
                for kc in range(NKC):
                    nc.sync.dma_start(out=xqT[:, kc, :], in_=xqt_d[ts(kc, 128), :])
                for mc in range(1, NKC):
                    nc.sync.dma_start(
                        out=wqc[:, mc],
                        in_=wqc_d[mc].rearrange("(kc p) c -> p kc c", p=128))
                for kc in range(NKC):
                    nc.sync.dma_start(out=xkvT[:, kc, :], in_=xkvt_d[ts(kc, 128), :])
                for mc in range(NKC):
                    nc.sync.dma_start(
                        out=wmk[:, mc],
                        in_=wmk_d[mc].rearrange("(kc p) c -> p kc c", p=128))
                for kc in range(NKC):
                    nc.sync.dma_start(out=wmv[:, kc, :], in_=wmv_d[ts(kc, 128), :])

                for jc in range(NJC):
                    nc.gpsimd.memset(vaug[:, jc, :, 64], 1.0)

                # q projection
                for mc in range(NKC):
                    ps = psum.tile([128, LQ], F32, tag="t1", bufs=4)
                    for kc in range(NKC):
                        nc.tensor.matmul(
                            ps, wqc[:, mc, kc, :], xqT[:, kc, :],
                            start=(kc == 0), stop=(kc == NKC - 1))
                    nc.scalar.activation(qT[:, mc, :], ps, CPY, scale=0.125)

                # k projection
                for mc in range(NKC):
                    for nh in range(2):
                        ps = psum.tile([128, 512], F32, tag="t1", bufs=4)
                        for kc in range(NKC):
                            nc.tensor.matmul(
                                ps, wmk[:, mc, kc, :], xkvT[:, kc, ts(nh, 512)],
                                start=(kc == 0), stop=(kc == NKC - 1))
                        if nh == 0:
                            nc.vector.tensor_copy(kT[:, mc, ts(nh, 512)], ps)
                        else:
                            nc.scalar.activation(kT[:, mc, ts(nh, 512)], ps, CPY)

                # v projection -> strided into vaug (cols 0..63 per head)
                for jc in range(NJC):
                    for nh in range(2):
                        ps = psum.tile([128, 512], F32, tag="t1", bufs=4)
                        for kc in range(NKC):
                            nc.tensor.matmul(
                                ps, xkvT[:, kc, ts(jc, 128)], wmv[:, kc, ts(nh, 512)],
                                start=(kc == 0), stop=(kc == NKC - 1))
                        dst = vaug[:, jc, nh * 8:(nh + 1) * 8, 0:64]
                        if nh == 0:
                            nc.vector.tensor_copy(dst, ps)
                        else:
                            nc.scalar.activation(dst, ps, CPY)

            # ================= phase C: attention =================
            with tc.tile_pool(name="phC", bufs=1) as phc:
                # gate-phase weights load during C
                wg_r = phc.tile([128, NKC, 2 * D], mdt)
                bg_sb = phc.tile([128, 2 * D], F32)
                for kc in range(NKC):
                    nc.sync.dma_start(out=wg_r[:, kc, :], in_=wg_d[ts(kc, 128), :])
                nc.sync.dma_start(out=bg_sb, in_=bg_d)

                for c in range(NKC):
                    for hh in range(2):
                        h = 2 * c + hh
                        srow_sb = phc.tile([128, SROW_W], mdt, tag="srow", bufs=4)
                        nc.sync.dma_start(out=srow_sb, in_=srow_d[h, :, :])
                        ps_h = psum.tile([65, LQ], F32, tag="psh", bufs=2)
                        for jc in range(NJC):
                            ps_s = psum.tile([128, LQ], F32, tag="t1", bufs=4)
                            nc.tensor.matmul(
                                ps_s, kT[ds2(hh), c, ts(jc, 128)], qT[ds2(hh), c, :],
                                start=True, stop=True)
                            wexp = phc.tile([128, LQ], mdt, tag="wexp", bufs=4)
                            nc.scalar.activation(wexp, ps_s, EXP)
                            wT = phc.tile([128, LQ], mdt, tag="wt", bufs=6)
                            C0 = 896 - jc * 128
                            nc.vector.tensor_tensor(
                                wT, wexp, srow_sb[:, C0:C0 + LQ], MUL)
                            nc.tensor.matmul(
                                ps_h, vaug[:, jc, h, :], wT,
                                start=(jc == 0), stop=(jc == NJC - 1))
                        rsb = phc.tile([128, LQ], mdt, tag="rsb", bufs=2)
                        with nc.allow_low_precision(reason="softmax recip bf16"):
                            nc.vector.reciprocal(rsb[64:65, :], ps_h[64:65, :])
                        rb_ps = psum.tile([64, LQ], F32, tag="rb", bufs=2)
                        nc.tensor.matmul(
                            rb_ps, onesc[64:65, :], rsb[64:65, :],
                            start=True, stop=True)
                        rb_sb = phc.tile([64, LQ], F32, tag="rbs", bufs=2)
                        nc.scalar.activation(rb_sb, rb_ps, CPY)
                        if hh == 0:
                            nc.vector.tensor_tensor(
                                attn[0:64, c, :], ps_h[0:64, :], rb_sb, MUL)
                        else:
                            todd = phc.tile([64, LQ], mdt, tag="todd", bufs=2)
                            nc.vector.tensor_tensor(todd, ps_h[0:64, :], rb_sb, MUL)
                            nc.sync.dma_start(out=attn[64:128, c, :], in_=todd)

                # ================= phase D: gate =================
                for ic in range(NIC):
                    for qa in range(2):
                        ps_b = psum.tile([128, 512], F32, tag="t1", bufs=4)
                        for kc in range(NKC):
                            nc.tensor.matmul(
                                ps_b, attn[:, kc, ts(ic, 128)],
                                wg_r[:, kc, slice(D + qa * 512, D + qa * 512 + 512)],
                                start=(kc == 0), stop=(kc == NKC - 1))
                        tb = phc.tile([128, 512], F32, tag="tb", bufs=2)
                        nc.vector.tensor_tensor(
                            tb, ps_b, bg_sb[:, D + qa * 512:D + qa * 512 + 512], ADD)
                        tsg = phc.tile([128, 512], F32, tag="tsg", bufs=2)
                        nc.scalar.activation(tsg, tb, SIG)

                        ps_a = psum.tile([128, 512], F32, tag="t1", bufs=4)
                        for kc in range(NKC):
                            nc.tensor.matmul(
                                ps_a, attn[:, kc, ts(ic, 128)],
                                wg_r[:, kc, ts(qa, 512)],
                                start=(kc == 0), stop=(kc == NKC - 1))
                        ta = phc.tile([128, 512], F32, tag="ta", bufs=2)
                        nc.vector.tensor_tensor(ta, ps_a, bg_sb[:, ts(qa, 512)], ADD)
                        outh = phc.tile([128, 512], F32, tag="outt", bufs=3)
                        nc.vector.tensor_tensor(outh, ta, tsg, MUL)
                        nc.sync.dma_start(
                            out=out_d[ts(ic, 128), ts(qa, 512)], in_=outh)

    nc.compile()
    return nc


# ======================= host side =======================

def _tisa_ebias(amp, off, sharp):
    d = np.arange(-(L - 1), L, dtype=np.float32)
    s = np.sum(
        amp[:, :, None].astype(np.float32)
        * np.exp(-np.abs(sharp)[:, :, None].astype(np.float32)
                 * (d[None, None, :] - off[:, :, None].astype(np.float32)) ** 2),
        axis=1, dtype=np.float32).astype(np.float32)
    return np.exp(s).astype(np.float32)


def make_host_inputs(inputs, cfg="bf16"):
    npdt = _NP["bf16"]
    x_q = np.asarray(inputs["x_q"])
    x_kv = np.asarray(inputs["x_kv"])
    wq = np.asarray(inputs["Wq"]).astype(npdt)
    wm = np.asarray(inputs["Wm"]).astype(npdt)
    wg = np.asarray(inputs["Wg"]).astype(npdt)
    bg = np.asarray(inputs["bg"]).astype(np.float32)

    ebias = _tisa_ebias(np.asarray(inputs["tisa_amp"]),
                        np.asarray(inputs["tisa_off"]),
                        np.asarray(inputs["tisa_sharp"]))

    p_i = np.arange(128)[:, None]
    m_i = np.arange(SROW_W)[None, :]
    srows = []
    for i_off in (0, 512):
        idx = p_i - m_i + (1919 - i_off)
        srows.append(np.ascontiguousarray(ebias[:, idx]).astype(npdt))

    # column-chunked (mc-major) layouts for early compute start
    wqc = np.ascontiguousarray(
        wq.reshape(D, NKC, 128).transpose(1, 0, 2))          # [mc, k, col]
    wmk = np.ascontiguousarray(
        wm[:, :D].reshape(D, NKC, 128).transpose(1, 0, 2))   # [mc, k, col]
    wmv = np.ascontiguousarray(wm[:, D:])                    # [k, v-col]

    bgrep = np.ascontiguousarray(np.broadcast_to(bg, (128, 2 * D))).astype(np.float32)

    in_maps = []
    for core in range(8):
        b, half = core // 2, core % 2
        in_maps.append({
            "xqt": np.ascontiguousarray(
                x_q[b, half * LQ:(half + 1) * LQ].T).astype(npdt),
            "xkvt": np.ascontiguousarray(x_kv[b].T).astype(npdt),
            "wqc": wqc, "wmk": wmk, "wmv": wmv, "wg": wg,
            "srow": srows[half],
            "bgrep": bgrep,
        })
    return in_maps


def assemble_output(results):
    out = np.empty((4, L, D), dtype=np.float32)
    for core in range(8):
        b, half = core // 2, core % 2
        out[b, half * LQ:(half + 1) * LQ] = results[core]["out"]
    return out


# ======================= public entry point =======================

_NC_CACHE = {}


def _get_nc(cfg):
    if cfg not in _NC_CACHE:
        _NC_CACHE[cfg] = build_nc(cfg)
    return _NC_CACHE[cfg]


def kernel(**inputs):
    """Full (unsharded) inputs -> full (4, 1024, 1024) float32 output.

    Shards over 8 NeuronCores: core = (batch, query-half). Host precomputes
    the TISA exp-bias lookup table and pre-transposes activations; all dense
    compute (projections, attention, gate) runs on-device in bf16 matmuls
    with fp32 accumulation.
    """
    from concourse.bass_utils import run_bass_kernel_spmd

    cfg = "bf16"
    nc = _get_nc(cfg)
    in_maps = make_host_inputs(inputs, cfg)
    res = run_bass_kernel_spmd(nc, in_maps, core_ids=list(range(8)))
    return assemble_output(res.results)


# revision 9
# speedup vs baseline: 1.2130x; 1.2030x over previous
"""CrossAttn + TISA bias kernel for TRN2, 8-core SPMD.

Sharding: core = (batch b = core//2, query half = core%2).
Each core computes the full kv projection for its batch (duplicated within
the pair) and its 512 query rows end-to-end. No collectives.

v2 vs v1:
  - softmax denominator fused into the attn matmul: stationary [v_h | 1]
    (M=65) so each wT tile streams through the PE once, not twice
  - per-head reciprocal broadcast via a K=1 ones matmul (bf16)
  - odd heads' normalized attn moved to partitions 64..127 with a small
    SBUF->SBUF DMA (engines cannot shift partitions)
  - weight DMAs reordered/column-chunked so the first q-proj matmul starts
    after ~0.4 MB instead of 9 MB
  - gate phase emits 512-col output halves, sigmoid path first, so the
    tail after the last matmul is short

Inputs arrive host-transposed: xqt/xkvt are [d_in, tokens].
  qT:   [d_out(part), i]   (scaled by 1/sqrt(Dh))
  kT:   [d_out(part), j]
  vaug: [j(part), jc, h, 65] = v columns 0..63, ones column 64
  S^T:  [j(part), i] = kT_h.T @ qT_h          (K=64)
  wT = exp(S^T) * srow[:, C:C+512]            (shifted exp-bias table slice)
  ps_h = [vaug_h]^T @ wT accumulated over jc  -> rows 0..63 attn, row 64 sums
  rb = ones1^T @ (1/sums)                     (K=1 PE broadcast)
  attn = ps_h * rb                            (per-head normalize)
  gate: attn.T @ Wg -> [i(part), 2048]; out = (a+bga)*sigmoid(b+bgb)
"""

import numpy as np
import ml_dtypes

import concourse.bacc as bacc
import concourse.mybir as mybir
import concourse.tile as tile
from concourse.bass import ts

L = 1024
D = 1024
H = 16
DH = 64
LQ = 512          # q rows per core
NIC = LQ // 128   # 4 i-chunks
NJC = L // 128    # 8 j-chunks
NKC = D // 128    # 8 d_model chunks
SROW_W = 1408
NUM_KERNELS = 21

F32 = mybir.dt.float32
BF16 = mybir.dt.bfloat16
EXP = mybir.ActivationFunctionType.Exp
SIG = mybir.ActivationFunctionType.Sigmoid
CPY = mybir.ActivationFunctionType.Copy
MUL = mybir.AluOpType.mult
ADD = mybir.AluOpType.add

_NP = {"f32": np.float32, "bf16": ml_dtypes.bfloat16}


def ds2(hh):
    return slice(hh * 64, hh * 64 + 64)


def build_nc(cfg="bf16"):
    mdt = BF16

    nc = bacc.Bacc("TRN2", target_bir_lowering=False, debug=False, num_devices=8)

    xqt_d = nc.dram_tensor("xqt", [D, LQ], mdt, kind="ExternalInput").ap()
    xkvt_d = nc.dram_tensor("xkvt", [D, L], mdt, kind="ExternalInput").ap()
    wqc_d = nc.dram_tensor("wqc", [NKC, D, 128], mdt, kind="ExternalInput").ap()
    wmk_d = nc.dram_tensor("wmk", [NKC, D, 128], mdt, kind="ExternalInput").ap()
    wmv_d = nc.dram_tensor("wmv", [D, D], mdt, kind="ExternalInput").ap()
    wg_d = nc.dram_tensor("wg", [D, 2 * D], mdt, kind="ExternalInput").ap()
    srow_d = nc.dram_tensor("srow", [H, 128, SROW_W], mdt, kind="ExternalInput").ap()
    bg_d = nc.dram_tensor("bgrep", [128, 2 * D], F32, kind="ExternalInput").ap()
    out_d = nc.dram_tensor("out", [LQ, D], F32, kind="ExternalOutput").ap()

    with tile.TileContext(nc) as tc:
        with (
            tc.tile_pool(name="const", bufs=1) as constp,
            tc.tile_pool(name="persist", bufs=1) as pers,
            tc.tile_pool(name="psum", bufs=1, space="PSUM") as psum,
        ):
            onesc = constp.tile([128, 64], mdt)
            nc.gpsimd.memset(onesc, 1.0)

            # PE warmup: keep the array busy during the initial input DMA so
            # the first real matmuls run at full clock (pstate ramp).
            dummy = constp.tile([128, 512], mdt)
            nc.gpsimd.memset(dummy, 0.0)
            for _ in range(7):
                ps_w = psum.tile([128, 512], F32, tag="t1", bufs=2)
                nc.tensor.matmul(ps_w, dummy[:, 0:128], dummy, start=True, stop=True)

            qT = pers.tile([128, NKC, LQ], mdt)        # [d_out, mc, i]
            kT = pers.tile([128, NKC, L], mdt)         # [d_out, mc, j]
            vaug = pers.tile([128, NJC, H, 65], mdt)   # [j, jc, h, v|1]
            attn = pers.tile([128, NKC, LQ], mdt)      # [d_model, chunk, i]

            # =========== phase B: projections ==========
            with tc.tile_pool(name="phB", bufs=1) as phb:
                wqc = phb.tile([128, NKC, NKC, 128], mdt)   # [k, mc, kc, col]
                xqT = phb.tile([128, NKC, LQ], mdt)         # [d_in, kc, i]
                xkvT = phb.tile([128, NKC, L], mdt)         # [d_in, kc, j]
                wmk = phb.tile([128, NKC, NKC, 128], mdt)
                wmv = phb.tile([128, NKC, D], mdt)          # [k, kc, v-cols]

                # Weights stream on the Pool SWDGE queue; activations on the
                # SP HWDGE queue. The two queues issue in parallel so the
                # first q-proj matmul starts after ~0.4 MB, not 9 MB.
                for mc in range(NKC):
                    nc.gpsimd.dma_start(
                        out=wqc[:, mc],
                        in_=wqc_d[mc].rearrange("(kc p) c -> p kc c", p=128))
                for mc in range(NKC):
                    nc.gpsimd.dma_start(
                        out=wmk[:, mc],
                        in_=wmk_d[mc].rearrange("(kc p) c -> p kc c", p=128))
                for kc in range(NKC):
                    nc.gpsimd.dma_start(out=wmv[:, kc, :], in_=wmv_d[ts(kc, 128), :])
                for kc in range(NKC):
                    nc.sync.dma_start(out=xqT[:, kc, :], in_=xqt_d[ts(kc, 128), :])
                for kc in range(NKC):
                    nc.sync.dma_start(out=xkvT[:, kc, :], in_=xkvt_d[ts(kc, 128), :])

                for jc in range(NJC):
                    nc.gpsimd.memset(vaug[:, jc, :, 64], 1.0)

                # q projection
                for mc in range(NKC):
                    ps = psum.tile([128, LQ], F32, tag="t1", bufs=2)
                    for kc in range(NKC):
                        nc.tensor.matmul(
                            ps, wqc[:, mc, kc, :], xqT[:, kc, :],
                            start=(kc == 0), stop=(kc == NKC - 1))
                    nc.scalar.activation(qT[:, mc, :], ps, CPY, scale=0.125)

                # k projection
                for mc in range(NKC):
                    for nh in range(2):
                        ps = psum.tile([128, 512], F32, tag="t1", bufs=2)
                        for kc in range(NKC):
                            nc.tensor.matmul(
                                ps, wmk[:, mc, kc, :], xkvT[:, kc, ts(nh, 512)],
                                start=(kc == 0), stop=(kc == NKC - 1))
                        if nh == 0:
                            nc.vector.tensor_copy(kT[:, mc, ts(nh, 512)], ps)
                        else:
                            nc.scalar.activation(kT[:, mc, ts(nh, 512)], ps, CPY)

                # v projection -> strided into vaug (cols 0..63 per head)
                for jc in range(NJC):
                    for nh in range(2):
                        ps = psum.tile([128, 512], F32, tag="t1", bufs=2)
                        for kc in range(NKC):
                            nc.tensor.matmul(
                                ps, xkvT[:, kc, ts(jc, 128)], wmv[:, kc, ts(nh, 512)],
                                start=(kc == 0), stop=(kc == NKC - 1))
                        dst = vaug[:, jc, nh * 8:(nh + 1) * 8, 0:64]
                        if nh == 0:
                            nc.vector.tensor_copy(dst, ps)
                        else:
                            nc.scalar.activation(dst, ps, CPY)

            # ================= phase C: attention =================
            with tc.tile_pool(name="phC", bufs=1) as phc:
                # gate-phase weights load during C (Pool SWDGE queue)
                wg_r = phc.tile([128, NKC, 2 * D], mdt)
                bg_sb = phc.tile([128, 2 * D], F32)
                for kc in range(NKC):
                    nc.gpsimd.dma_start(out=wg_r[:, kc, :], in_=wg_d[ts(kc, 128), :])
                nc.gpsimd.dma_start(out=bg_sb, in_=bg_d)

                # srow prefetch, 4 heads deep (SP queue drains early)
                srow_tiles = {}
                for h in range(4):
                    srow_tiles[h] = phc.tile([128, SROW_W], mdt, tag="srow", bufs=4, name=f"srow{h}")
                    nc.sync.dma_start(out=srow_tiles[h], in_=srow_d[h, :, :])

                for c in range(NKC):
                    for hh in range(2):
                        h = 2 * c + hh
                        srow_sb = srow_tiles.pop(h)
                        if h + 4 < H:
                            srow_tiles[h + 4] = phc.tile(
                                [128, SROW_W], mdt, tag="srow", bufs=4,
                                name=f"srow{h + 4}")
                            nc.sync.dma_start(
                                out=srow_tiles[h + 4], in_=srow_d[h + 4, :, :])
                        ps_h = psum.tile([65, LQ], F32, tag="psh", bufs=2)
                        for jp in range(NJC // 2):
                            # scores for a jc pair share a 2-bank psum so one
                            # EXP covers 1024 cols (halves Act fixed cost)
                            ps_s = psum.tile([128, 2 * LQ], F32, tag="ss", bufs=2)
                            for t in range(2):
                                jc = 2 * jp + t
                                nc.tensor.matmul(
                                    ps_s[:, ts(t, LQ)],
                                    kT[ds2(hh), c, ts(jc, 128)], qT[ds2(hh), c, :],
                                    start=True, stop=True)
                            wexp = phc.tile([128, 2 * LQ], mdt, tag="wexp", bufs=3)
                            nc.scalar.activation(wexp, ps_s, EXP)
                            for t in range(2):
                                jc = 2 * jp + t
                                wT = phc.tile([128, LQ], mdt, tag="wt", bufs=6)
                                C0 = 896 - jc * 128
                                nc.vector.tensor_tensor(
                                    wT, wexp[:, ts(t, LQ)],
                                    srow_sb[:, C0:C0 + LQ], MUL)
                                nc.tensor.matmul(
                                    ps_h, vaug[:, jc, h, :], wT,
                                    start=(jc == 0), stop=(jc == NJC - 1))
                        rsb = phc.tile([128, LQ], mdt, tag="rsb", bufs=2)
                        with nc.allow_low_precision(reason="softmax recip bf16"):
                            nc.vector.reciprocal(rsb[64:65, :], ps_h[64:65, :])
                        rb_ps = psum.tile([64, LQ], F32, tag="t1", bufs=2)
                        nc.tensor.matmul(
                            rb_ps, onesc[64:65, :], rsb[64:65, :],
                            start=True, stop=True)
                        rb_sb = phc.tile([64, LQ], F32, tag="rbs", bufs=2)
                        nc.vector.tensor_copy(rb_sb, rb_ps)
                        if hh == 0:
                            nc.vector.tensor_tensor(
                                attn[0:64, c, :], ps_h[0:64, :], rb_sb, MUL)
                        else:
                            todd = phc.tile([64, LQ], mdt, tag="todd", bufs=2)
                            nc.vector.tensor_tensor(todd, ps_h[0:64, :], rb_sb, MUL)
                            nc.sync.dma_start(out=attn[64:128, c, :], in_=todd)

                # ================= phase D: gate =================
                for ic in range(NIC):
                    for qa in range(2):
                        ps_b = psum.tile([128, 512], F32, tag="t1", bufs=2)
                        for kc in range(NKC):
                            nc.tensor.matmul(
                                ps_b, attn[:, kc, ts(ic, 128)],
                                wg_r[:, kc, slice(D + qa * 512, D + qa * 512 + 512)],
                                start=(kc == 0), stop=(kc == NKC - 1))
                        tb = phc.tile([128, 512], F32, tag="tb", bufs=2)
                        nc.vector.tensor_tensor(
                            tb, ps_b, bg_sb[:, D + qa * 512:D + qa * 512 + 512], ADD)
                        tsg = phc.tile([128, 512], F32, tag="tsg", bufs=2)
                        nc.scalar.activation(tsg, tb, SIG)

                        ps_a = psum.tile([128, 512], F32, tag="t1", bufs=2)
                        for kc in range(NKC):
                            nc.tensor.matmul(
                                ps_a, attn[:, kc, ts(ic, 128)],
                                wg_r[:, kc, ts(qa, 512)],
                                start=(kc == 0), stop=(kc == NKC - 1))
                        last = (ic == NIC - 1) and (qa == 1)
                        if not last:
                            ta = phc.tile([128, 512], F32, tag="ta", bufs=2)
                            nc.vector.tensor_tensor(
                                ta, ps_a, bg_sb[:, ts(qa, 512)], ADD)
                            outh = phc.tile([128, 512], F32, tag="outt", bufs=3)
                            nc.vector.tensor_tensor(outh, ta, tsg, MUL)
                            nc.sync.dma_start(
                                out=out_d[ts(ic, 128), ts(qa, 512)], in_=outh)
                        else:
                            # split the final chunk so the post-matmul tail
                            # (adds, mul, DMA) pipelines in 256-col pieces
                            for half in range(2):
                                sl = slice(half * 256, half * 256 + 256)
                                ta = phc.tile([128, 256], F32, tag="ta2", bufs=2)
                                nc.vector.tensor_tensor(
                                    ta, ps_a[:, sl],
                                    bg_sb[:, qa * 512 + half * 256:
                                          qa * 512 + half * 256 + 256], ADD)
                                outh = phc.tile([128, 256], F32, tag="outt2", bufs=2)
                                nc.vector.tensor_tensor(
                                    outh, ta, tsg[:, sl], MUL)
                                nc.sync.dma_start(
                                    out=out_d[ts(ic, 128),
                                              qa * 512 + half * 256:
                                              qa * 512 + half * 256 + 256],
                                    in_=outh)

    nc.compile()
    return nc


# ======================= host side =======================

def _tisa_ebias(amp, off, sharp):
    d = np.arange(-(L - 1), L, dtype=np.float32)
    s = np.sum(
        amp[:, :, None].astype(np.float32)
        * np.exp(-np.abs(sharp)[:, :, None].astype(np.float32)
                 * (d[None, None, :] - off[:, :, None].astype(np.float32)) ** 2),
        axis=1, dtype=np.float32).astype(np.float32)
    return np.exp(s).astype(np.float32)


def make_host_inputs(inputs, cfg="bf16"):
    npdt = _NP["bf16"]
    x_q = np.asarray(inputs["x_q"])
    x_kv = np.asarray(inputs["x_kv"])
    wq = np.asarray(inputs["Wq"]).astype(npdt)
    wm = np.asarray(inputs["Wm"]).astype(npdt)
    wg = np.asarray(inputs["Wg"]).astype(npdt)
    bg = np.asarray(inputs["bg"]).astype(np.float32)

    ebias = _tisa_ebias(np.asarray(inputs["tisa_amp"]),
                        np.asarray(inputs["tisa_off"]),
                        np.asarray(inputs["tisa_sharp"]))

    p_i = np.arange(128)[:, None]
    m_i = np.arange(SROW_W)[None, :]
    srows = []
    for i_off in (0, 512):
        idx = p_i - m_i + (1919 - i_off)
        srows.append(np.ascontiguousarray(ebias[:, idx]).astype(npdt))

    # column-chunked (mc-major) layouts for early compute start
    wqc = np.ascontiguousarray(
        wq.reshape(D, NKC, 128).transpose(1, 0, 2))          # [mc, k, col]
    wmk = np.ascontiguousarray(
        wm[:, :D].reshape(D, NKC, 128).transpose(1, 0, 2))   # [mc, k, col]
    wmv = np.ascontiguousarray(wm[:, D:])                    # [k, v-col]

    bgrep = np.ascontiguousarray(np.broadcast_to(bg, (128, 2 * D))).astype(np.float32)

    in_maps = []
    for core in range(8):
        b, half = core // 2, core % 2
        in_maps.append({
            "xqt": np.ascontiguousarray(
                x_q[b, half * LQ:(half + 1) * LQ].T).astype(npdt),
            "xkvt": np.ascontiguousarray(x_kv[b].T).astype(npdt),
            "wqc": wqc, "wmk": wmk, "wmv": wmv, "wg": wg,
            "srow": srows[half],
            "bgrep": bgrep,
        })
    return in_maps


def assemble_output(results):
    out = np.empty((4, L, D), dtype=np.float32)
    for core in range(8):
        b, half = core // 2, core % 2
        out[b, half * LQ:(half + 1) * LQ] = results[core]["out"]
    return out


# ======================= public entry point =======================

_NC_CACHE = {}


def _get_nc(cfg):
    if cfg not in _NC_CACHE:
        _NC_CACHE[cfg] = build_nc(cfg)
    return _NC_CACHE[cfg]


def kernel(**inputs):
    """Full (unsharded) inputs -> full (4, 1024, 1024) float32 output.

    Shards over 8 NeuronCores: core = (batch, query-half). Host precomputes
    the TISA exp-bias lookup table and pre-transposes activations; all dense
    compute (projections, attention, gate) runs on-device in bf16 matmuls
    with fp32 accumulation.
    """
    from concourse.bass_utils import run_bass_kernel_spmd

    cfg = "bf16"
    nc = _get_nc(cfg)
    in_maps = make_host_inputs(inputs, cfg)
    res = run_bass_kernel_spmd(nc, in_maps, core_ids=list(range(8)))
    return assemble_output(res.results)
